# revision 37
# baseline (speedup 1.0000x reference)
import os
import numpy as np
from contextlib import ExitStack

import concourse.bass as bass
import concourse.bacc as bacc
import concourse.mybir as mybir
import concourse.tile as tile
from concourse import masks
from concourse.bass_utils import run_bass_kernel_spmd

NCORES = 8
B = 8
C = 256
HW = 1024
PL = HW // NCORES  # 128 query positions per core
NCH = 8            # column chunks of the Gram sweep
CW = HW * B // NCH  # 1024 (j,d) columns per chunk

F32 = mybir.dt.float32
F32R = mybir.dt.float32r


def build_nc(use_f32r=True, finalize=True):
    MD = F32R if use_f32r else F32

    # Bacc (not plain Bass): its compile() pass legalizes sync — multi-wait
    # matmuls move waits onto LdWeights, drains become EventSemaphores.
    nc = bacc.Bacc(None, target_bir_lowering=False)

    # Inputs (per-core identical except xm):
    #   xg: replicated g-input, layout [kc, c_local, j*8+d]
    #   xm: per-core slice, layout [kc, c_local, k*128+p_local]
    #   wg/wm: w_g.T / w_mask.T chunked on contraction axis
    xg_h = nc.declare_dram_parameter("xg", [2, 128, 8192], MD, isOutput=False)
    xm_h = nc.declare_dram_parameter("xm", [2, 128, 1024], MD, isOutput=False)
    wg_h = nc.declare_dram_parameter("wg", [2, 128, 256], MD, isOutput=False)
    wm_h = nc.declare_dram_parameter("wm", [2, 128, 256], MD, isOutput=False)
    out_h = nc.declare_dram_parameter("out", [B, C, PL], F32, isOutput=True)

    with (
        tile.TileContext(nc) as tc,
        ExitStack() as ctx,
    ):
        sb = ctx.enter_context(tc.tile_pool(name="sb", bufs=1))
        dram = ctx.enter_context(tc.tile_pool(name="dram", bufs=1, space="DRAM"))
        # padded to 4KB/32KB: tiny CC payloads fail at runtime
        r_loc = dram.tile([1024], F32, name="r_loc", tag="r_loc")
        r_all = dram.tile([8192], F32, name="r_all", tag="r_all", addr_space="Shared")
        r_locW = dram.tile([1024], F32, name="r_locW", tag="r_locW")
        r_allW = dram.tile([8192], F32, name="r_allW", tag="r_allW", addr_space="Shared")
        em_d = dram.tile([1024], F32, name="em_d", tag="em_d")

        # 6 banks of Gram tiles (3-deep ring decouples PE/DVE ping-pong) +
        # 2 banks for gt/conv staging; tail transposes borrow p1-ring slices
        ps_gram = ctx.enter_context(tc.tile_pool(name="ps_gram", bufs=3, space="PSUM"))
        ps_gt = ctx.enter_context(tc.tile_pool(name="ps_gt", bufs=2, space="PSUM"))

        wgt = [sb.tile([128, 256], MD, name=f"wg{c}", tag=f"wg{c}") for c in range(2)]
        wmt = [sb.tile([128, 256], MD, name=f"wm{c}", tag=f"wm{c}") for c in range(2)]
        xmt = [sb.tile([128, 1024], MD, name=f"xm{c}", tag=f"xm{c}") for c in range(2)]
        xgt = [[sb.tile([128, 2048], MD, name=f"xg{c}_{q}", tag=f"xg{c}_{q}") for q in range(4)] for c in range(2)]
        gm = [sb.tile([128, 1024], MD, name=f"gm{c}", tag=f"gm{c}") for c in range(2)]
        gt = [sb.tile([128, 8192], MD, name=f"g{c}", tag=f"g{c}") for c in range(2)]
        gmaxt = [sb.tile([128, 1024], F32, name=f"gmax{t}", tag=f"gmax{t}") for t in range(8)]
        conv = [sb.tile([128, 1024], F32, name=f"conv{c}", tag=f"conv{c}") for c in range(2)]
        emB = sb.tile([128, 1024], F32, name="emB", tag="emB")
        ident = sb.tile([128, 128], F32, name="ident", tag="ident")
        ident8 = sb.tile([8, 8], F32, name="ident8", tag="ident8")
        ones1 = sb.tile([1, 128], F32, name="ones1", tag="ones1")
        rsb = sb.tile([128, 8], F32, name="rsb", tag="rsb")
        rm8 = sb.tile([8, 128], F32, name="rm8", tag="rm8")
        em = sb.tile([8, 128], F32, name="em", tag="em")
        rsmall = sb.tile([8, 128], F32, name="rsmall", tag="rsmall")
        rt2 = sb.tile([8, 16], F32, name="rt2", tag="rt2")
        corr = sb.tile([8, 8], F32, name="corr", tag="corr")
        prod = sb.tile([8, 8], F32, name="prod", tag="prod")
        lmax = sb.tile([8, 1], F32, name="lmax", tag="lmax")
        negl = sb.tile([8, 1], F32, name="negl", tag="negl")
        lsum = sb.tile([8, 1], F32, name="lsum", tag="lsum")
        gmax = sb.tile([8, 1], F32, name="gmax", tag="gmax")
        negg = sb.tile([8, 1], F32, name="negg", tag="negg")
        gsum = sb.tile([8, 1], F32, name="gsum", tag="gsum")
        rinv = sb.tile([8, 1], F32, name="rinv", tag="rinv")
        myc = sb.tile([8, 1], F32, name="myc", tag="myc")
        sc = sb.tile([8, 1], F32, name="sc", tag="sc")
        scRow = sb.tile([1, 8], F32, name="scRow", tag="scRow")
        scP = sb.tile([128, 8], F32, name="scP", tag="scP")
        convEm = [gmaxt[0], gmaxt[1]]   # reused after rowsums complete
        outsb = [gmaxt[2], gmaxt[3]]

        # ---- input DMAs ----
        # sync queue feeds the first PE work (gm needs wg+xm); chunk-0 of the
        # replicated input goes first on the gpsimd queue so gt(0) isn't
        # stuck behind the whole 8MB load. wm is only needed ~100us in.
        def xg_dma(eng, kc, g):
            q, half = g // 2, g % 2
            eng.dma_start(
                out=xgt[kc][q][:, half * 1024:(half + 1) * 1024],
                in_=xg_h[kc, :, g * 1024:(g + 1) * 1024],
            )

        nc.sync.dma_start(out=wgt[0][:], in_=wg_h[0])
        nc.sync.dma_start(out=wgt[1][:], in_=wg_h[1])
        nc.sync.dma_start(out=xmt[0][:, 0:512], in_=xm_h[0, :, 0:512])
        nc.sync.dma_start(out=xmt[1][:, 0:512], in_=xm_h[1, :, 0:512])
        nc.sync.dma_start(out=xmt[0][:, 512:1024], in_=xm_h[0, :, 512:1024])
        nc.sync.dma_start(out=xmt[1][:, 512:1024], in_=xm_h[1, :, 512:1024])
        xg_dma(nc.gpsimd, 1, 0)
        xg_dma(nc.gpsimd, 0, 0)
        for g in range(1, NCH):
            xg_dma(nc.sync, 0, g)
            xg_dma(nc.gpsimd, 1, g)
        for cc in range(2):
            nc.sync.dma_start(out=wmt[cc][:], in_=wm_h[cc])
        # warm up the CC stream: throwaway AllGathers absorb the
        # collective's first-op setup + rendezvous cost during the compute
        # region, so the real exchange later runs warm (two warmups in case
        # op state is double-buffered). The doorbell occupies the gpsimd
        # queue for ~10us, so they go AFTER every input DMA on that queue.
        nc.gpsimd.collective_compute(
            "AllGather",
            mybir.AluOpType.bypass,
            replica_groups=[list(range(NCORES))],
            ins=[r_locW[:].opt()],
            outs=[r_allW[:].opt()],
        )

        # ---- constants (gpsimd, after its input DMAs — needed late) ----
        masks.make_identity(nc, ident[:])
        masks.make_identity(nc, ident8[:])
        nc.gpsimd.memset(ones1[:], 1.0)
        nc.gpsimd.memset(rsmall[:], 0.0)

        # ---- gm = w_g @ x_mine (per-core g, k-major cols) ----
        for co in range(2):
            for h in range(2):
                pt = ps_gt.tile([128, 512], F32, name="p1", tag="p1")
                for kc in range(2):
                    nc.tensor.matmul(
                        out=pt[:],
                        lhsT=wgt[kc][:, co * 128:(co + 1) * 128],
                        rhs=xmt[kc][:, h * 512:(h + 1) * 512],
                        start=(kc == 0),
                        stop=(kc == 1),
                    )
                nc.scalar.copy(out=gm[co][:, h * 512:(h + 1) * 512], in_=pt[:])

        def gt_chunk(g):
            # gt[:, chunk g] = w_g @ x_all chunk ((j,d)-interleaved cols)
            q, half = g // 2, g % 2
            for co in range(2):
                for h in range(2):
                    pt = ps_gt.tile([128, 512], F32, name="p1", tag="p1")
                    for kc in range(2):
                        nc.tensor.matmul(
                            out=pt[:],
                            lhsT=wgt[kc][:, co * 128:(co + 1) * 128],
                            rhs=xgt[kc][q][:, half * 1024 + h * 512:half * 1024 + (h + 1) * 512],
                            start=(kc == 0),
                            stop=(kc == 1),
                        )
                    nc.scalar.copy(
                        out=gt[co][:, g * 1024 + h * 512:g * 1024 + (h + 1) * 512],
                        in_=pt[:],
                    )

        gt_chunk(0)

        # ---- main sweep: Gram chunk g + grouped d-max; prefetch gt chunk g+1 ----
        for g in range(NCH):
            if g + 1 < NCH:
                gt_chunk(g + 1)
            for t in range(8):
                pt2 = ps_gram.tile([128, 1024], F32, name="p2", tag="p2")
                for kc in range(2):
                    for h in range(2):
                        nc.tensor.matmul(
                            out=pt2[:, h * 512:(h + 1) * 512],
                            lhsT=gm[kc][:, t * 128:(t + 1) * 128],
                            rhs=gt[kc][:, g * 1024 + h * 512:g * 1024 + (h + 1) * 512],
                            start=(kc == 0),
                            stop=(kc == 1),
                        )
                nc.vector.reduce_max(
                    out=gmaxt[t][:, g * 128:(g + 1) * 128],
                    in_=pt2[:].rearrange("p (j e) -> p j e", e=8),
                    axis=mybir.AxisListType.X,
                )
                if g == NCH - 1:
                    # row sums: t<7 on the scalar engine (runs behind the
                    # remaining reduces), the last one on DVE so the stats
                    # transpose isn't stuck behind a serial ACT tail
                    if t < 7:
                        nc.scalar.activation(
                            out=emB[:], in_=gmaxt[t][:],
                            func=mybir.ActivationFunctionType.Copy,
                            accum_out=rsb[:, t:t + 1],
                        )
                    else:
                        nc.vector.reduce_sum(
                            out=rsb[:, t:t + 1], in_=gmaxt[t][:],
                            axis=mybir.AxisListType.X,
                        )

        # ---- transpose rsb [128,8] -> [8,128] on the PE ----
        # (borrows a p1-ring PSUM slot; gt staging is finished by now)
        trsb_ps = ps_gt.tile([128, 512], F32, name="p1", tag="p1")
        nc.tensor.transpose(out=trsb_ps[0:8, 0:128], in_=rsb[:], identity=ident[:])
        nc.vector.tensor_copy(rm8[:], trsb_ps[0:8, 0:128])

        # ---- local softmax stats (two-phase softmax) ----
        nc.vector.reduce_max(out=lmax[:], in_=rm8[:], axis=mybir.AxisListType.X)
        nc.vector.tensor_scalar_mul(out=negl[:], in0=lmax[:], scalar1=-1.0 / 128.0)
        nc.scalar.activation(
            out=em[:], in_=rm8[:], func=mybir.ActivationFunctionType.Exp,
            bias=negl[:], scale=1.0 / 128.0, accum_out=lsum[:],
        )
        nc.vector.tensor_copy(rsmall[:, 0:1], lmax[:])
        nc.vector.tensor_copy(rsmall[:, 1:2], lsum[:])

        # ---- AllGather 16 floats (lmax|lsum per batch) across cores ----
        core_ids = list(range(NCORES))
        nc.gpsimd.dma_start(
            out=r_loc[:].rearrange("(k c) -> k c", c=128), in_=rsmall[:],
        )
        nc.gpsimd.collective_compute(
            "AllGather",
            mybir.AluOpType.bypass,
            replica_groups=[core_ids],
            ins=[r_loc[:].opt()],
            outs=[r_all[:].opt()],
        )

        # ---- work overlapped with the collective ----
        # conv = w_mask @ x_mine
        for co in range(2):
            for h in range(2):
                pt = ps_gt.tile([128, 512], F32, name="p1", tag="p1")
                for kc in range(2):
                    nc.tensor.matmul(
                        out=pt[:],
                        lhsT=wmt[kc][:, co * 128:(co + 1) * 128],
                        rhs=xmt[kc][:, h * 512:(h + 1) * 512],
                        start=(kc == 0),
                        stop=(kc == 1),
                    )
                nc.scalar.copy(out=conv[co][:, h * 512:(h + 1) * 512], in_=pt[:])
        # broadcast em over partitions: emB[p, k*128+m] = em[k, m]
        nc.sync.dma_start(out=em_d[:].rearrange("(k p) -> k p", k=8), in_=em[:])
        ed = em_d[:]
        bcast = bass.AP(tensor=ed.tensor, offset=ed.offset, ap=[[0, 128], [1, 1024]])
        nc.sync.dma_start(out=emB[:], in_=bcast)
        for co in range(2):
            nc.vector.tensor_mul(out=convEm[co][:], in0=conv[co][:], in1=emB[:])

        # ---- gather stats, combine ----
        # r_all layout: [r*1024 + k*128 + c], c in 0..1 -> rt2[k, r*2+c]
        ra = r_all[:]
        nc.gpsimd.dma_start(
            out=rt2[:].rearrange("k (r c) -> k r c", c=2),
            in_=bass.AP(tensor=ra.tensor, offset=ra.offset,
                        ap=[[128, 8], [1024, 8], [1, 2]]),
        )
        a = rt2[:]
        lmaxl = bass.AP(tensor=a.tensor, offset=a.offset, ap=[a.ap[0], [2, 8]])
        lsuml = bass.AP(tensor=a.tensor, offset=a.offset + 1, ap=[a.ap[0], [2, 8]])
        nc.vector.reduce_max(out=gmax[:], in_=lmaxl, axis=mybir.AxisListType.X)
        nc.vector.tensor_scalar_mul(out=negg[:], in0=gmax[:], scalar1=-1.0 / 128.0)
        nc.scalar.activation(
            out=corr[:], in_=lmaxl, func=mybir.ActivationFunctionType.Exp,
            bias=negg[:], scale=1.0 / 128.0,
        )
        nc.vector.tensor_mul(out=prod[:], in0=corr[:], in1=lsuml)
        nc.vector.reduce_sum(out=gsum[:], in_=prod[:], axis=mybir.AxisListType.X)
        nc.vector.reciprocal(out=rinv[:], in_=gsum[:])
        nc.scalar.activation(
            out=myc[:], in_=lmax[:], func=mybir.ActivationFunctionType.Exp,
            bias=negg[:], scale=1.0 / 128.0,
        )
        nc.vector.tensor_mul(out=sc[:], in0=myc[:], in1=rinv[:])

        # ---- replicate sc over partitions via PE: scP[p, k] = sc[k] ----
        scpad = ps_gt.tile([128, 512], F32, name="p1", tag="p1")
        nc.tensor.transpose(out=scpad[0:1, 0:8], in_=sc[:], identity=ident8[:])
        nc.vector.tensor_copy(scRow[:], scpad[0:1, 0:8])
        nc.tensor.matmul(out=scpad[:, 8:16], lhsT=ones1[:], rhs=scRow[:],
                         start=True, stop=True)
        nc.vector.tensor_copy(scP[:], scpad[:, 8:16])

        # ---- final: out = conv * em * sc[k]; scale and ship each
        # (co, k-half) as it completes so DMA transfers overlap the
        # remaining muls. DVE muls are ~1.7x faster than ACT's, so it
        # takes 3 of every 4. ----
        for half in range(2):
            for co in range(2):
                for kk in range(4):
                    k = half * 4 + kk
                    dst = outsb[co][:, k * 128:(k + 1) * 128]
                    src = convEm[co][:, k * 128:(k + 1) * 128]
                    if kk == 3:
                        nc.scalar.mul(dst, src, scP[:, k:k + 1])
                    else:
                        nc.vector.tensor_scalar_mul(out=dst, in0=src, scalar1=scP[:, k:k + 1])
                eng = nc.sync if co == 0 else nc.gpsimd
                eng.dma_start(
                    out=out_h[half * 4:(half + 1) * 4, co * 128:(co + 1) * 128, :]
                    .rearrange("k co p -> co k p"),
                    in_=outsb[co][:, half * 512:(half + 1) * 512]
                    .rearrange("co (k p) -> co k p", k=4),
                )

    if finalize:
        nc.finalize()
    return nc


def _prep_inputs(x, w_g, w_mask):
    xr = x.reshape(B, C, HW)
    # xg cols: j*8+d  (j = pixel, d = batch), rows c
    xg = np.ascontiguousarray(xr.transpose(1, 2, 0)).reshape(2, 128, 8192)
    wg = np.ascontiguousarray(w_g.T).reshape(2, 128, 256)
    wm = np.ascontiguousarray(w_mask.T).reshape(2, 128, 256)
    in_maps = []
    for r in range(NCORES):
        xs = xr[:, :, r * PL:(r + 1) * PL]
        # xm cols: k*128 + p_local, rows c
        xm = np.ascontiguousarray(xs.transpose(1, 0, 2)).reshape(2, 128, 1024)
        in_maps.append({"xg": xg, "xm": xm, "wg": wg, "wm": wm})
    return in_maps


def kernel(**inputs):
    x = np.ascontiguousarray(inputs["x"], dtype=np.float32)
    w_g = np.ascontiguousarray(inputs["w_g"], dtype=np.float32)
    w_mask = np.ascontiguousarray(inputs["w_mask"], dtype=np.float32)

    in_maps = _prep_inputs(x, w_g, w_mask)
    nc = build_nc(use_f32r=os.environ.get("KERNEL_NO_F32R", "0") != "1")
    trace = os.environ.get("KERNEL_TRACE", "0") == "1"
    res = run_bass_kernel_spmd(nc, in_maps, list(range(NCORES)), trace=trace)
    globals()["_last_exec_time_ns"] = getattr(res, "exec_time_ns", None)
    outs = [res.results[i]["out"] for i in range(NCORES)]
    return np.concatenate(outs, axis=2).reshape(B, C, 32, 32).astype(np.float32)


# revision 39
# speedup vs baseline: 1.0108x; 1.0108x over previous
import os
import numpy as np
from contextlib import ExitStack

import concourse.bass as bass
import concourse.bacc as bacc
import concourse.mybir as mybir
import concourse.tile as tile
from concourse import masks
from concourse.bass_utils import run_bass_kernel_spmd

NCORES = 8
B = 8
C = 256
HW = 1024
PL = HW // NCORES  # 128 query positions per core
NCH = 8            # column chunks of the Gram sweep
CW = HW * B // NCH  # 1024 (j,d) columns per chunk

F32 = mybir.dt.float32
F32R = mybir.dt.float32r


def build_nc(use_f32r=True, finalize=True):
    MD = F32R if use_f32r else F32

    # Bacc (not plain Bass): its compile() pass legalizes sync — multi-wait
    # matmuls move waits onto LdWeights, drains become EventSemaphores.
    nc = bacc.Bacc(None, target_bir_lowering=False)

    # Inputs (per-core identical except xm):
    #   xg: replicated g-input, layout [kc, c_local, j*8+d]
    #   xm: per-core slice, layout [kc, c_local, k*128+p_local]
    #   wg/wm: w_g.T / w_mask.T chunked on contraction axis
    xg_h = nc.declare_dram_parameter("xg", [2, 128, 8192], MD, isOutput=False)
    xm_h = nc.declare_dram_parameter("xm", [2, 128, 1024], MD, isOutput=False)
    wg_h = nc.declare_dram_parameter("wg", [2, 128, 256], MD, isOutput=False)
    wm_h = nc.declare_dram_parameter("wm", [2, 128, 256], MD, isOutput=False)
    out_h = nc.declare_dram_parameter("out", [B, C, PL], F32, isOutput=True)

    with (
        tile.TileContext(nc) as tc,
        ExitStack() as ctx,
    ):
        sb = ctx.enter_context(tc.tile_pool(name="sb", bufs=1))
        dram = ctx.enter_context(tc.tile_pool(name="dram", bufs=1, space="DRAM"))
        # padded to 4KB/32KB: tiny CC payloads fail at runtime
        r_loc = dram.tile([1024], F32, name="r_loc", tag="r_loc")
        r_all = dram.tile([8192], F32, name="r_all", tag="r_all", addr_space="Shared")
        r_locW = dram.tile([1024], F32, name="r_locW", tag="r_locW")
        r_allW = dram.tile([8192], F32, name="r_allW", tag="r_allW", addr_space="Shared")
        em_d = dram.tile([1024], F32, name="em_d", tag="em_d")

        # 6 banks of Gram tiles (3-deep ring decouples PE/DVE ping-pong) +
        # 2 banks for gt/conv staging; tail transposes borrow p1-ring slices
        ps_gram = ctx.enter_context(tc.tile_pool(name="ps_gram", bufs=3, space="PSUM"))
        ps_gt = ctx.enter_context(tc.tile_pool(name="ps_gt", bufs=2, space="PSUM"))

        wgt = [sb.tile([128, 256], MD, name=f"wg{c}", tag=f"wg{c}") for c in range(2)]
        wmt = [sb.tile([128, 256], MD, name=f"wm{c}", tag=f"wm{c}") for c in range(2)]
        xmt = [sb.tile([128, 1024], MD, name=f"xm{c}", tag=f"xm{c}") for c in range(2)]
        xgt = [[sb.tile([128, 2048], MD, name=f"xg{c}_{q}", tag=f"xg{c}_{q}") for q in range(4)] for c in range(2)]
        gm = [sb.tile([128, 1024], MD, name=f"gm{c}", tag=f"gm{c}") for c in range(2)]
        gt = [sb.tile([128, 8192], MD, name=f"g{c}", tag=f"g{c}") for c in range(2)]
        gmaxt = [sb.tile([128, 1024], F32, name=f"gmax{t}", tag=f"gmax{t}") for t in range(8)]
        conv = [sb.tile([128, 1024], F32, name=f"conv{c}", tag=f"conv{c}") for c in range(2)]
        emB = sb.tile([128, 1024], F32, name="emB", tag="emB")
        ident = sb.tile([128, 128], F32, name="ident", tag="ident")
        ident8 = sb.tile([8, 8], F32, name="ident8", tag="ident8")
        ones1 = sb.tile([1, 128], F32, name="ones1", tag="ones1")
        rsb = sb.tile([128, 8], F32, name="rsb", tag="rsb")
        rm8 = sb.tile([8, 128], F32, name="rm8", tag="rm8")
        em = sb.tile([8, 128], F32, name="em", tag="em")
        rsmall = sb.tile([8, 128], F32, name="rsmall", tag="rsmall")
        rt2 = sb.tile([8, 16], F32, name="rt2", tag="rt2")
        corr = sb.tile([8, 8], F32, name="corr", tag="corr")
        prod = sb.tile([8, 8], F32, name="prod", tag="prod")
        lmax = sb.tile([8, 1], F32, name="lmax", tag="lmax")
        negl = sb.tile([8, 1], F32, name="negl", tag="negl")
        lsum = sb.tile([8, 1], F32, name="lsum", tag="lsum")
        gmax = sb.tile([8, 1], F32, name="gmax", tag="gmax")
        negg = sb.tile([8, 1], F32, name="negg", tag="negg")
        gsum = sb.tile([8, 1], F32, name="gsum", tag="gsum")
        rinv = sb.tile([8, 1], F32, name="rinv", tag="rinv")
        myc = sb.tile([8, 1], F32, name="myc", tag="myc")
        sc = sb.tile([8, 1], F32, name="sc", tag="sc")
        scRow = sb.tile([1, 8], F32, name="scRow", tag="scRow")
        scP = sb.tile([128, 8], F32, name="scP", tag="scP")
        convEm = [gmaxt[0], gmaxt[1]]   # reused after rowsums complete
        outsb = [gmaxt[2], gmaxt[3]]

        # ---- input DMAs ----
        # sync queue feeds the first PE work (gm needs wg+xm); chunk-0 of the
        # replicated input goes first on the gpsimd queue so gt(0) isn't
        # stuck behind the whole 8MB load. wm is only needed ~100us in.
        def xg_dma(eng, kc, g):
            q, half = g // 2, g % 2
            eng.dma_start(
                out=xgt[kc][q][:, half * 1024:(half + 1) * 1024],
                in_=xg_h[kc, :, g * 1024:(g + 1) * 1024],
            )

        nc.sync.dma_start(out=wgt[0][:], in_=wg_h[0])
        nc.sync.dma_start(out=xmt[0][:], in_=xm_h[0])
        nc.sync.dma_start(out=wgt[1][:], in_=wg_h[1])
        nc.sync.dma_start(out=xmt[1][:], in_=xm_h[1])
        xg_dma(nc.gpsimd, 1, 0)
        xg_dma(nc.gpsimd, 0, 0)
        for g in range(1, NCH):
            xg_dma(nc.sync, 0, g)
            xg_dma(nc.gpsimd, 1, g)
        for cc in range(2):
            nc.sync.dma_start(out=wmt[cc][:], in_=wm_h[cc])
        # warm up the CC stream: throwaway AllGathers absorb the
        # collective's first-op setup + rendezvous cost during the compute
        # region, so the real exchange later runs warm (two warmups in case
        # op state is double-buffered). The doorbell occupies the gpsimd
        # queue for ~10us, so they go AFTER every input DMA on that queue.
        nc.gpsimd.collective_compute(
            "AllGather",
            mybir.AluOpType.bypass,
            replica_groups=[list(range(NCORES))],
            ins=[r_locW[:].opt()],
            outs=[r_allW[:].opt()],
        )

        # ---- constants (gpsimd, after its input DMAs — needed late) ----
        masks.make_identity(nc, ident[:])
        masks.make_identity(nc, ident8[:])
        nc.gpsimd.memset(ones1[:], 1.0)
        nc.gpsimd.memset(rsmall[:], 0.0)

        # ---- gm = w_g @ x_mine (per-core g, k-major cols) ----
        for co in range(2):
            for h in range(2):
                pt = ps_gt.tile([128, 512], F32, name="p1", tag="p1")
                for kc in range(2):
                    nc.tensor.matmul(
                        out=pt[:],
                        lhsT=wgt[kc][:, co * 128:(co + 1) * 128],
                        rhs=xmt[kc][:, h * 512:(h + 1) * 512],
                        start=(kc == 0),
                        stop=(kc == 1),
                    )
                nc.scalar.copy(out=gm[co][:, h * 512:(h + 1) * 512], in_=pt[:])

        def gt_chunk(g):
            # gt[:, chunk g] = w_g @ x_all chunk ((j,d)-interleaved cols)
            q, half = g // 2, g % 2
            for co in range(2):
                for h in range(2):
                    pt = ps_gt.tile([128, 512], F32, name="p1", tag="p1")
                    for kc in range(2):
                        nc.tensor.matmul(
                            out=pt[:],
                            lhsT=wgt[kc][:, co * 128:(co + 1) * 128],
                            rhs=xgt[kc][q][:, half * 1024 + h * 512:half * 1024 + (h + 1) * 512],
                            start=(kc == 0),
                            stop=(kc == 1),
                        )
                    nc.scalar.copy(
                        out=gt[co][:, g * 1024 + h * 512:g * 1024 + (h + 1) * 512],
                        in_=pt[:],
                    )

        gt_chunk(0)

        # ---- main sweep: Gram chunk g + grouped d-max; prefetch gt chunk g+1 ----
        for g in range(NCH):
            if g + 1 < NCH:
                gt_chunk(g + 1)
            for t in range(8):
                pt2 = ps_gram.tile([128, 1024], F32, name="p2", tag="p2")
                for kc in range(2):
                    for h in range(2):
                        nc.tensor.matmul(
                            out=pt2[:, h * 512:(h + 1) * 512],
                            lhsT=gm[kc][:, t * 128:(t + 1) * 128],
                            rhs=gt[kc][:, g * 1024 + h * 512:g * 1024 + (h + 1) * 512],
                            start=(kc == 0),
                            stop=(kc == 1),
                        )
                nc.vector.reduce_max(
                    out=gmaxt[t][:, g * 128:(g + 1) * 128],
                    in_=pt2[:].rearrange("p (j e) -> p j e", e=8),
                    axis=mybir.AxisListType.X,
                )
                if g == NCH - 1:
                    # row sums: t<7 on the scalar engine (runs behind the
                    # remaining reduces), the last one on DVE so the stats
                    # transpose isn't stuck behind a serial ACT tail
                    if t < 7:
                        nc.scalar.activation(
                            out=emB[:], in_=gmaxt[t][:],
                            func=mybir.ActivationFunctionType.Copy,
                            accum_out=rsb[:, t:t + 1],
                        )
                    else:
                        nc.vector.reduce_sum(
                            out=rsb[:, t:t + 1], in_=gmaxt[t][:],
                            axis=mybir.AxisListType.X,
                        )

        # ---- transpose rsb [128,8] -> [8,128] on the PE ----
        # (borrows a p1-ring PSUM slot; gt staging is finished by now)
        trsb_ps = ps_gt.tile([128, 512], F32, name="p1", tag="p1")
        nc.tensor.transpose(out=trsb_ps[0:8, 0:128], in_=rsb[:], identity=ident[:])
        nc.vector.tensor_copy(rm8[:], trsb_ps[0:8, 0:128])

        # ---- local softmax stats (two-phase softmax) ----
        nc.vector.reduce_max(out=lmax[:], in_=rm8[:], axis=mybir.AxisListType.X)
        nc.vector.tensor_scalar_mul(out=negl[:], in0=lmax[:], scalar1=-1.0 / 128.0)
        nc.scalar.activation(
            out=em[:], in_=rm8[:], func=mybir.ActivationFunctionType.Exp,
            bias=negl[:], scale=1.0 / 128.0, accum_out=lsum[:],
        )
        nc.vector.tensor_copy(rsmall[:, 0:1], lmax[:])
        nc.vector.tensor_copy(rsmall[:, 1:2], lsum[:])

        # ---- AllGather 16 floats (lmax|lsum per batch) across cores ----
        core_ids = list(range(NCORES))
        nc.gpsimd.dma_start(
            out=r_loc[:].rearrange("(k c) -> k c", c=128), in_=rsmall[:],
        )
        nc.gpsimd.collective_compute(
            "AllGather",
            mybir.AluOpType.bypass,
            replica_groups=[core_ids],
            ins=[r_loc[:].opt()],
            outs=[r_all[:].opt()],
        )

        # ---- work overlapped with the collective ----
        # conv = w_mask @ x_mine
        for co in range(2):
            for h in range(2):
                pt = ps_gt.tile([128, 512], F32, name="p1", tag="p1")
                for kc in range(2):
                    nc.tensor.matmul(
                        out=pt[:],
                        lhsT=wmt[kc][:, co * 128:(co + 1) * 128],
                        rhs=xmt[kc][:, h * 512:(h + 1) * 512],
                        start=(kc == 0),
                        stop=(kc == 1),
                    )
                nc.scalar.copy(out=conv[co][:, h * 512:(h + 1) * 512], in_=pt[:])
        # broadcast em over partitions: emB[p, k*128+m] = em[k, m]
        nc.sync.dma_start(out=em_d[:].rearrange("(k p) -> k p", k=8), in_=em[:])
        ed = em_d[:]
        bcast = bass.AP(tensor=ed.tensor, offset=ed.offset, ap=[[0, 128], [1, 1024]])
        nc.sync.dma_start(out=emB[:], in_=bcast)
        for co in range(2):
            nc.vector.tensor_mul(out=convEm[co][:], in0=conv[co][:], in1=emB[:])

        # ---- gather stats, combine ----
        # r_all layout: [r*1024 + k*128 + c], c in 0..1 -> rt2[k, r*2+c]
        ra = r_all[:]
        nc.gpsimd.dma_start(
            out=rt2[:].rearrange("k (r c) -> k r c", c=2),
            in_=bass.AP(tensor=ra.tensor, offset=ra.offset,
                        ap=[[128, 8], [1024, 8], [1, 2]]),
        )
        a = rt2[:]
        lmaxl = bass.AP(tensor=a.tensor, offset=a.offset, ap=[a.ap[0], [2, 8]])
        lsuml = bass.AP(tensor=a.tensor, offset=a.offset + 1, ap=[a.ap[0], [2, 8]])
        nc.vector.reduce_max(out=gmax[:], in_=lmaxl, axis=mybir.AxisListType.X)
        nc.vector.tensor_scalar_mul(out=negg[:], in0=gmax[:], scalar1=-1.0 / 128.0)
        nc.scalar.activation(
            out=corr[:], in_=lmaxl, func=mybir.ActivationFunctionType.Exp,
            bias=negg[:], scale=1.0 / 128.0,
        )
        nc.vector.tensor_mul(out=prod[:], in0=corr[:], in1=lsuml)
        nc.vector.reduce_sum(out=gsum[:], in_=prod[:], axis=mybir.AxisListType.X)
        nc.vector.reciprocal(out=rinv[:], in_=gsum[:])
        nc.scalar.activation(
            out=myc[:], in_=lmax[:], func=mybir.ActivationFunctionType.Exp,
            bias=negg[:], scale=1.0 / 128.0,
        )
        nc.vector.tensor_mul(out=sc[:], in0=myc[:], in1=rinv[:])

        # ---- replicate sc over partitions via PE: scP[p, k] = sc[k] ----
        scpad = ps_gt.tile([128, 512], F32, name="p1", tag="p1")
        nc.tensor.transpose(out=scpad[0:1, 0:8], in_=sc[:], identity=ident8[:])
        nc.vector.tensor_copy(scRow[:], scpad[0:1, 0:8])
        nc.tensor.matmul(out=scpad[:, 8:16], lhsT=ones1[:], rhs=scRow[:],
                         start=True, stop=True)
        nc.vector.tensor_copy(scP[:], scpad[:, 8:16])

        # ---- final: out = conv * em * sc[k], DMA out ----
        for co in range(2):
            for k in range(8):
                dst = outsb[co][:, k * 128:(k + 1) * 128]
                src = convEm[co][:, k * 128:(k + 1) * 128]
                if k % 2 == 0:
                    nc.scalar.mul(dst, src, scP[:, k:k + 1])
                else:
                    nc.vector.tensor_scalar_mul(out=dst, in0=src, scalar1=scP[:, k:k + 1])
        for co, eng in ((0, nc.sync), (1, nc.gpsimd)):
            eng.dma_start(
                out=out_h[:, co * 128:(co + 1) * 128, :].rearrange("k co p -> co k p"),
                in_=outsb[co][:].rearrange("co (k p) -> co k p", k=8),
            )

    if finalize:
        nc.finalize()
    return nc


def _prep_inputs(x, w_g, w_mask):
    xr = x.reshape(B, C, HW)
    # xg cols: j*8+d  (j = pixel, d = batch), rows c
    xg = np.ascontiguousarray(xr.transpose(1, 2, 0)).reshape(2, 128, 8192)
    wg = np.ascontiguousarray(w_g.T).reshape(2, 128, 256)
    wm = np.ascontiguousarray(w_mask.T).reshape(2, 128, 256)
    in_maps = []
    for r in range(NCORES):
        xs = xr[:, :, r * PL:(r + 1) * PL]
        # xm cols: k*128 + p_local, rows c
        xm = np.ascontiguousarray(xs.transpose(1, 0, 2)).reshape(2, 128, 1024)
        in_maps.append({"xg": xg, "xm": xm, "wg": wg, "wm": wm})
    return in_maps


def kernel(**inputs):
    x = np.ascontiguousarray(inputs["x"], dtype=np.float32)
    w_g = np.ascontiguousarray(inputs["w_g"], dtype=np.float32)
    w_mask = np.ascontiguousarray(inputs["w_mask"], dtype=np.float32)

    in_maps = _prep_inputs(x, w_g, w_mask)
    nc = build_nc(use_f32r=os.environ.get("KERNEL_NO_F32R", "0") != "1")
    trace = os.environ.get("KERNEL_TRACE", "0") == "1"
    res = run_bass_kernel_spmd(nc, in_maps, list(range(NCORES)), trace=trace)
    globals()["_last_exec_time_ns"] = getattr(res, "exec_time_ns", None)
    outs = [res.results[i]["out"] for i in range(NCORES)]
    return np.concatenate(outs, axis=2).reshape(B, C, 32, 32).astype(np.float32)


# revision 46
# speedup vs baseline: 1.1230x; 1.1110x over previous
import os
import numpy as np
from contextlib import ExitStack

import concourse.bass as bass
import concourse.bacc as bacc
import concourse.mybir as mybir
import concourse.tile as tile
from concourse import masks
from concourse.bass_utils import run_bass_kernel_spmd

NCORES = 8
B = 8
C = 256
HW = 1024
PL = HW // NCORES  # 128 query positions per core
NCH = 8            # column chunks of the Gram sweep
CW = HW * B // NCH  # 1024 (j,d) columns per chunk

F32 = mybir.dt.float32
F32R = mybir.dt.float32r


def build_nc(use_f32r=True, finalize=True):
    MD = F32R if use_f32r else F32

    # Bacc (not plain Bass): its compile() pass legalizes sync — multi-wait
    # matmuls move waits onto LdWeights, drains become EventSemaphores.
    nc = bacc.Bacc(None, target_bir_lowering=False)

    # Inputs (per-core identical except xm):
    #   xg: replicated g-input, layout [kc, c_local, j*8+d]
    #   xm: per-core slice, layout [kc, c_local, k*128+p_local]
    #   wg/wm: w_g.T / w_mask.T chunked on contraction axis
    xg_h = nc.declare_dram_parameter("xg", [2, 128, 8192], MD, isOutput=False)
    xm_h = nc.declare_dram_parameter("xm", [2, 128, 1024], MD, isOutput=False)
    wg_h = nc.declare_dram_parameter("wg", [2, 128, 256], MD, isOutput=False)
    wm_h = nc.declare_dram_parameter("wm", [2, 128, 256], MD, isOutput=False)
    out_h = nc.declare_dram_parameter("out", [B, C, PL], F32, isOutput=True)

    with (
        tile.TileContext(nc) as tc,
        ExitStack() as ctx,
    ):
        sb = ctx.enter_context(tc.tile_pool(name="sb", bufs=1))
        dram = ctx.enter_context(tc.tile_pool(name="dram", bufs=1, space="DRAM"))
        # padded to 4KB/32KB: tiny CC payloads fail at runtime
        r_loc = dram.tile([1024], F32, name="r_loc", tag="r_loc")
        r_all = dram.tile([8192], F32, name="r_all", tag="r_all", addr_space="Shared")
        r_locW = dram.tile([1024], F32, name="r_locW", tag="r_locW")
        r_allW = dram.tile([8192], F32, name="r_allW", tag="r_allW", addr_space="Shared")
        em_d = dram.tile([1024], F32, name="em_d", tag="em_d")

        # 6 banks of Gram tiles (3-deep ring decouples PE/DVE ping-pong) +
        # 2 banks for gt/conv staging; tail transposes borrow p1-ring slices
        ps_gram = ctx.enter_context(tc.tile_pool(name="ps_gram", bufs=3, space="PSUM"))
        ps_gt = ctx.enter_context(tc.tile_pool(name="ps_gt", bufs=2, space="PSUM"))

        wgt = [sb.tile([128, 256], MD, name=f"wg{c}", tag=f"wg{c}") for c in range(2)]
        wmt = [sb.tile([128, 256], MD, name=f"wm{c}", tag=f"wm{c}") for c in range(2)]
        xmt = [sb.tile([128, 1024], MD, name=f"xm{c}", tag=f"xm{c}") for c in range(2)]
        xgt = [[sb.tile([128, 2048], MD, name=f"xg{c}_{q}", tag=f"xg{c}_{q}") for q in range(4)] for c in range(2)]
        gm = [sb.tile([128, 1024], MD, name=f"gm{c}", tag=f"gm{c}") for c in range(2)]
        gt = [sb.tile([128, 8192], MD, name=f"g{c}", tag=f"g{c}") for c in range(2)]
        gmaxt = [sb.tile([128, 1024], F32, name=f"gmax{t}", tag=f"gmax{t}") for t in range(8)]
        conv = [sb.tile([128, 1024], F32, name=f"conv{c}", tag=f"conv{c}") for c in range(2)]
        emB = sb.tile([128, 1024], F32, name="emB", tag="emB")
        ident = sb.tile([128, 128], F32, name="ident", tag="ident")
        ident8 = sb.tile([8, 8], F32, name="ident8", tag="ident8")
        ones1 = sb.tile([1, 128], F32, name="ones1", tag="ones1")
        rsb = sb.tile([128, 8], F32, name="rsb", tag="rsb")
        rm8 = sb.tile([8, 128], F32, name="rm8", tag="rm8")
        em = sb.tile([8, 128], F32, name="em", tag="em")
        rsmall = sb.tile([8, 128], F32, name="rsmall", tag="rsmall")
        rt2 = sb.tile([8, 16], F32, name="rt2", tag="rt2")
        corr = sb.tile([8, 8], F32, name="corr", tag="corr")
        prod = sb.tile([8, 8], F32, name="prod", tag="prod")
        lmax = sb.tile([8, 1], F32, name="lmax", tag="lmax")
        negl = sb.tile([8, 1], F32, name="negl", tag="negl")
        lsum = sb.tile([8, 1], F32, name="lsum", tag="lsum")
        gmax = sb.tile([8, 1], F32, name="gmax", tag="gmax")
        negg = sb.tile([8, 1], F32, name="negg", tag="negg")
        gsum = sb.tile([8, 1], F32, name="gsum", tag="gsum")
        rinv = sb.tile([8, 1], F32, name="rinv", tag="rinv")
        myc = sb.tile([8, 1], F32, name="myc", tag="myc")
        sc = sb.tile([8, 1], F32, name="sc", tag="sc")
        scRow = sb.tile([1, 8], F32, name="scRow", tag="scRow")
        scP = sb.tile([128, 8], F32, name="scP", tag="scP")
        convEm = [gmaxt[0], gmaxt[1]]   # reused after rowsums complete
        outsb = [gmaxt[2], gmaxt[3]]

        # ---- input DMAs ----
        # sync queue feeds the first PE work (gm needs wg+xm); chunk-0 of the
        # replicated input goes first on the gpsimd queue so gt(0) isn't
        # stuck behind the whole 8MB load. wm is only needed ~100us in.
        def xg_dma(eng, kc, g):
            q, half = g // 2, g % 2
            eng.dma_start(
                out=xgt[kc][q][:, half * 1024:(half + 1) * 1024],
                in_=xg_h[kc, :, g * 1024:(g + 1) * 1024],
            )

        nc.sync.dma_start(out=wgt[0][:], in_=wg_h[0])
        nc.sync.dma_start(out=xmt[0][:], in_=xm_h[0])
        nc.sync.dma_start(out=wgt[1][:], in_=wg_h[1])
        nc.sync.dma_start(out=xmt[1][:], in_=xm_h[1])
        xg_dma(nc.gpsimd, 1, 0)
        xg_dma(nc.gpsimd, 0, 0)
        for g in range(1, NCH):
            xg_dma(nc.sync, 0, g)
            xg_dma(nc.gpsimd, 1, g)
        for cc in range(2):
            nc.sync.dma_start(out=wmt[cc][:], in_=wm_h[cc])
        # warm up the CC stream: throwaway AllGathers absorb the
        # collective's first-op setup + rendezvous cost during the compute
        # region, so the real exchange later runs warm (two warmups in case
        # op state is double-buffered). The doorbell occupies the gpsimd
        # queue for ~10us, so they go AFTER every input DMA on that queue.
        nc.gpsimd.collective_compute(
            "AllGather",
            mybir.AluOpType.bypass,
            replica_groups=[list(range(NCORES))],
            ins=[r_locW[:].opt()],
            outs=[r_allW[:].opt()],
        )

        # ---- constants (gpsimd, after its input DMAs — needed late) ----
        masks.make_identity(nc, ident[:])
        masks.make_identity(nc, ident8[:])
        nc.gpsimd.memset(ones1[:], 1.0)
        nc.gpsimd.memset(rsmall[:], 0.0)

        # ---- gm = w_g @ x_mine (per-core g, k-major cols) ----
        for co in range(2):
            for h in range(2):
                pt = ps_gt.tile([128, 512], F32, name="p1", tag="p1")
                for kc in range(2):
                    nc.tensor.matmul(
                        out=pt[:],
                        lhsT=wgt[kc][:, co * 128:(co + 1) * 128],
                        rhs=xmt[kc][:, h * 512:(h + 1) * 512],
                        start=(kc == 0),
                        stop=(kc == 1),
                    )
                nc.scalar.copy(out=gm[co][:, h * 512:(h + 1) * 512], in_=pt[:])

        def gt_chunk(g):
            # gt[:, chunk g] = w_g @ x_all chunk ((j,d)-interleaved cols)
            q, half = g // 2, g % 2
            for co in range(2):
                for h in range(2):
                    pt = ps_gt.tile([128, 512], F32, name="p1", tag="p1")
                    for kc in range(2):
                        nc.tensor.matmul(
                            out=pt[:],
                            lhsT=wgt[kc][:, co * 128:(co + 1) * 128],
                            rhs=xgt[kc][q][:, half * 1024 + h * 512:half * 1024 + (h + 1) * 512],
                            start=(kc == 0),
                            stop=(kc == 1),
                        )
                    nc.scalar.copy(
                        out=gt[co][:, g * 1024 + h * 512:g * 1024 + (h + 1) * 512],
                        in_=pt[:],
                    )

        gt_chunk(0)

        # ---- main sweep: Gram chunk g + grouped d-max; prefetch gt chunk g+1 ----
        for g in range(NCH):
            if g + 1 < NCH:
                gt_chunk(g + 1)
            for t in range(8):
                pt2 = ps_gram.tile([128, 1024], F32, name="p2", tag="p2")
                for kc in range(2):
                    for h in range(2):
                        nc.tensor.matmul(
                            out=pt2[:, h * 512:(h + 1) * 512],
                            lhsT=gm[kc][:, t * 128:(t + 1) * 128],
                            rhs=gt[kc][:, g * 1024 + h * 512:g * 1024 + (h + 1) * 512],
                            start=(kc == 0),
                            stop=(kc == 1),
                        )
                nc.vector.reduce_max(
                    out=gmaxt[t][:, g * 128:(g + 1) * 128],
                    in_=pt2[:].rearrange("p (j e) -> p j e", e=8),
                    axis=mybir.AxisListType.X,
                )
                if g == NCH - 1:
                    # row sums: t<7 on the scalar engine (runs behind the
                    # remaining reduces), the last one on DVE so the stats
                    # transpose isn't stuck behind a serial ACT tail
                    if t < 7:
                        nc.scalar.activation(
                            out=emB[:], in_=gmaxt[t][:],
                            func=mybir.ActivationFunctionType.Copy,
                            accum_out=rsb[:, t:t + 1],
                        )
                    else:
                        nc.vector.reduce_sum(
                            out=rsb[:, t:t + 1], in_=gmaxt[t][:],
                            axis=mybir.AxisListType.X,
                        )

        # ---- transpose rsb [128,8] -> [8,128] on the PE ----
        # (borrows a p1-ring PSUM slot; gt staging is finished by now)
        trsb_ps = ps_gt.tile([128, 512], F32, name="p1", tag="p1")
        nc.tensor.transpose(out=trsb_ps[0:8, 0:128], in_=rsb[:], identity=ident[:])
        nc.vector.tensor_copy(rm8[:], trsb_ps[0:8, 0:128])

        # ---- local softmax stats (two-phase softmax) ----
        nc.vector.reduce_max(out=lmax[:], in_=rm8[:], axis=mybir.AxisListType.X)
        nc.vector.tensor_scalar_mul(out=negl[:], in0=lmax[:], scalar1=-1.0 / 128.0)
        nc.scalar.activation(
            out=em[:], in_=rm8[:], func=mybir.ActivationFunctionType.Exp,
            bias=negl[:], scale=1.0 / 128.0, accum_out=lsum[:],
        )
        nc.vector.tensor_copy(rsmall[:, 0:1], lmax[:])
        nc.vector.tensor_copy(rsmall[:, 1:2], lsum[:])

        # ---- AllGather 16 floats (lmax|lsum per batch) across cores ----
        core_ids = list(range(NCORES))
        nc.gpsimd.dma_start(
            out=r_loc[:].rearrange("(k c) -> k c", c=128), in_=rsmall[:],
        )
        nc.gpsimd.collective_compute(
            "AllGather",
            mybir.AluOpType.bypass,
            replica_groups=[core_ids],
            ins=[r_loc[:].opt()],
            outs=[r_all[:].opt()],
        )

        # ---- work overlapped with the collective ----
        # conv = w_mask @ x_mine
        for co in range(2):
            for h in range(2):
                pt = ps_gt.tile([128, 512], F32, name="p1", tag="p1")
                for kc in range(2):
                    nc.tensor.matmul(
                        out=pt[:],
                        lhsT=wmt[kc][:, co * 128:(co + 1) * 128],
                        rhs=xmt[kc][:, h * 512:(h + 1) * 512],
                        start=(kc == 0),
                        stop=(kc == 1),
                    )
                nc.scalar.copy(out=conv[co][:, h * 512:(h + 1) * 512], in_=pt[:])
        # broadcast em over partitions: emB[p, k*128+m] = em[k, m]
        nc.sync.dma_start(out=em_d[:].rearrange("(k p) -> k p", k=8), in_=em[:])
        ed = em_d[:]
        bcast = bass.AP(tensor=ed.tensor, offset=ed.offset, ap=[[0, 128], [1, 1024]])
        nc.sync.dma_start(out=emB[:], in_=bcast)
        for co in range(2):
            nc.vector.tensor_mul(out=convEm[co][:], in0=conv[co][:], in1=emB[:])

        # ---- gather stats, combine ----
        # r_all layout: [r*1024 + k*128 + c], c in 0..1 -> rt2[k, r*2+c]
        ra = r_all[:]
        nc.gpsimd.dma_start(
            out=rt2[:].rearrange("k (r c) -> k r c", c=2),
            in_=bass.AP(tensor=ra.tensor, offset=ra.offset,
                        ap=[[128, 8], [1024, 8], [1, 2]]),
        )
        a = rt2[:]
        lmaxl = bass.AP(tensor=a.tensor, offset=a.offset, ap=[a.ap[0], [2, 8]])
        lsuml = bass.AP(tensor=a.tensor, offset=a.offset + 1, ap=[a.ap[0], [2, 8]])
        nc.vector.reduce_max(out=gmax[:], in_=lmaxl, axis=mybir.AxisListType.X)
        nc.vector.tensor_scalar_mul(out=negg[:], in0=gmax[:], scalar1=-1.0 / 128.0)
        nc.scalar.activation(
            out=corr[:], in_=lmaxl, func=mybir.ActivationFunctionType.Exp,
            bias=negg[:], scale=1.0 / 128.0,
        )
        nc.vector.tensor_mul(out=prod[:], in0=corr[:], in1=lsuml)
        nc.vector.reduce_sum(out=gsum[:], in_=prod[:], axis=mybir.AxisListType.X)
        nc.vector.reciprocal(out=rinv[:], in_=gsum[:])
        nc.scalar.activation(
            out=myc[:], in_=lmax[:], func=mybir.ActivationFunctionType.Exp,
            bias=negg[:], scale=1.0 / 128.0,
        )
        nc.vector.tensor_mul(out=sc[:], in0=myc[:], in1=rinv[:])

        # ---- replicate sc over partitions via PE: scP[p, k] = sc[k] ----
        scpad = ps_gt.tile([128, 512], F32, name="p1", tag="p1")
        nc.tensor.transpose(out=scpad[0:1, 0:8], in_=sc[:], identity=ident8[:])
        nc.vector.tensor_copy(scRow[:], scpad[0:1, 0:8])
        nc.tensor.matmul(out=scpad[:, 8:16], lhsT=ones1[:], rhs=scRow[:],
                         start=True, stop=True)
        nc.vector.tensor_copy(scP[:], scpad[:, 8:16])

        # ---- final: out = conv * em * sc[k], DMA out ----
        for co in range(2):
            for k in range(8):
                dst = outsb[co][:, k * 128:(k + 1) * 128]
                src = convEm[co][:, k * 128:(k + 1) * 128]
                if k % 2 == 0:
                    nc.scalar.mul(dst, src, scP[:, k:k + 1])
                else:
                    nc.vector.tensor_scalar_mul(out=dst, in0=src, scalar1=scP[:, k:k + 1])
        for co, eng in ((0, nc.sync), (1, nc.gpsimd)):
            eng.dma_start(
                out=out_h[:, co * 128:(co + 1) * 128, :].rearrange("k co p -> co k p"),
                in_=outsb[co][:].rearrange("co (k p) -> co k p", k=8),
            )

    if finalize:
        nc.finalize()
    return nc


def _prep_inputs(x, w_g, w_mask):
    xr = x.reshape(B, C, HW)
    # xg cols: j*8+d  (j = pixel, d = batch), rows c
    xg = np.ascontiguousarray(xr.transpose(1, 2, 0)).reshape(2, 128, 8192)
    wg = np.ascontiguousarray(w_g.T).reshape(2, 128, 256)
    wm = np.ascontiguousarray(w_mask.T).reshape(2, 128, 256)
    in_maps = []
    for r in range(NCORES):
        xs = xr[:, :, r * PL:(r + 1) * PL]
        # xm cols: k*128 + p_local, rows c
        xm = np.ascontiguousarray(xs.transpose(1, 0, 2)).reshape(2, 128, 1024)
        in_maps.append({"xg": xg, "xm": xm, "wg": wg, "wm": wm})
    return in_maps


def kernel(**inputs):
    x = np.ascontiguousarray(inputs["x"], dtype=np.float32)
    w_g = np.ascontiguousarray(inputs["w_g"], dtype=np.float32)
    w_mask = np.ascontiguousarray(inputs["w_mask"], dtype=np.float32)

    in_maps = _prep_inputs(x, w_g, w_mask)
    nc = build_nc(use_f32r=os.environ.get("KERNEL_NO_F32R", "0") != "1")
    trace = os.environ.get("KERNEL_TRACE", "0") == "1"
    res = run_bass_kernel_spmd(nc, in_maps, list(range(NCORES)), trace=trace)
    globals()["_last_exec_time_ns"] = getattr(res, "exec_time_ns", None)
    outs = [res.results[i]["out"] for i in range(NCORES)]
    return np.concatenate(outs, axis=2).reshape(B, C, 32, 32).astype(np.float32)
